# revision 1
# baseline (speedup 1.0000x reference)
"""Causal MHA (B=4, S=2048, D=1024, H=16) on 8 NeuronCores.

Sharding: tensor-parallel over heads — 2 heads per core. Each core computes
Q/K/V projections for its 2 heads over the whole batch, causal flash-style
attention, and its partial output projection; the host sums the 8 partials.

Precision: the softmax here is near-argmax (scores have std ~1e3), so the
score path (Q/K projections and Q.K^T) runs in native fp32 on the PE array;
V, probs, and the output projection run in bf16. Softmax is per-block online
(block max + per-block exp from PSUM + fused correction*1/l multiply), so the
PV matmul directly produces normalized attn^T with both heads column-packed.
"""

import numpy as np

B, S, D = 4, 2048, 1024
H, E = 16, 64
NCORES = 8
HPC = H // NCORES       # heads per core = 2
E2 = HPC * E            # 128
P = 128
KB = 512                # score block columns (k per psum tile)
SBLK = 512              # s-block for Q/K projection rhs tiles
NDT = D // P            # 8 d-tiles
NQT = S // P            # 16 q-tiles per batch
NST = S // P            # 16 s-tiles per batch
FMIN = -3.0e38

_BUILT = None


def _apply_drain_patch():
    """This walrus build rejects >1 sync-wait on a CTRL (Drain) instruction.
    Split the TileContext exit-drain waits across several drain instructions."""
    import concourse.tile as tile
    import concourse.mybir as mybir
    from concourse.vector_clock import ScopedClock

    if getattr(tile.TileContext, "_drain_patch_applied", False):
        return

    def _drain_and_barrier(self, tick_clock, wait_clock):
        nc = self.nc
        drain_inst = nc.sync.drain()
        wait_clock.add_sem_waits(
            drain_inst.ins, ScopedClock({None: tick_clock.global_clock})
        )
        si = drain_inst.ins.sync_info
        if si is not None and len(si.on_wait) > 1:
            waits = list(si.on_wait)
            del si.on_wait[1:]
            for w in waits[1:]:
                d2 = nc.sync.drain()
                d2.ins.sync_info = mybir.SyncInfo(on_wait=[w], on_update=[])
        nc.all_engine_barrier()
        popped = nc._tile_sem_poison_stack.pop()
        assert popped is self._sem_poison
        nc.clear_and_free_semaphores(list(self.sems.allocated().values()))
        nc.all_engine_barrier()

    tile.TileContext._drain_and_barrier = _drain_and_barrier
    tile.TileContext._drain_patch_applied = True


def _split_multiwaits(nc, max_waits=1):
    """This walrus build rejects instructions carrying more than ~1 sync-wait.
    Hoist extra waits onto single-wait NoOps on the same engine, placed just
    before the instruction (sequencers execute in order, so semantics hold)."""
    import concourse.mybir as mybir

    n_new = 0
    for f in nc.m.functions:
        for blk in f.blocks:
            insts = list(blk.instructions)
            if not any(
                getattr(i, "sync_info", None) is not None
                and len(i.sync_info.on_wait) > max_waits
                for i in insts
            ):
                continue
            out = []
            for inst in insts:
                si = getattr(inst, "sync_info", None)
                if si is not None and len(si.on_wait) > max_waits:
                    extra = list(si.on_wait[max_waits:])
                    del si.on_wait[max_waits:]
                    for w in extra:
                        n_new += 1
                        out.append(mybir.InstNoOp(
                            name=f"splitw-{n_new}",
                            sync_info=mybir.SyncInfo(on_wait=[w], on_update=[]),
                            engine=inst.engine,
                            bass_nofuse=True,
                        ))
                out.append(inst)
            blk.instructions[:] = out
    return n_new


def build_nc(reps=1):
    import concourse.bass as bass
    import concourse.mybir as mybir
    import concourse.tile as tile

    _apply_drain_patch()
    dt = mybir.dt
    Exp = mybir.ActivationFunctionType.Exp

    nc = bass.Bass("TRN2", target_bir_lowering=False, debug=False)

    x_d = nc.dram_tensor("x", [B, D, S], dt.float32, kind="ExternalInput").ap()
    xh_d = nc.dram_tensor("xh", [B, D, S], dt.bfloat16, kind="ExternalInput").ap()
    qw_d = nc.dram_tensor("qw", [D, E2], dt.float32, kind="ExternalInput").ap()
    kw_d = nc.dram_tensor("kw", [D, E2], dt.float32, kind="ExternalInput").ap()
    vw_d = nc.dram_tensor("vw", [D, E2], dt.bfloat16, kind="ExternalInput").ap()
    ow_d = nc.dram_tensor("ow", [E2, D], dt.bfloat16, kind="ExternalInput").ap()
    me_d = nc.dram_tensor("me", [P, 4, KB], dt.bfloat16, kind="ExternalInput").ap()
    out_d = nc.dram_tensor("out", [B, S, D], dt.float32, kind="ExternalOutput").ap()

    with tile.TileContext(nc) as tc:
        with (
            tc.tile_pool(name="const", bufs=1) as cpool,
            tc.tile_pool(name="big", bufs=2) as big,
            tc.tile_pool(name="xs", bufs=2) as xs,
            tc.tile_pool(name="xsh", bufs=2) as xsh,
            tc.tile_pool(name="vt", bufs=1) as vt_p,
            tc.tile_pool(name="prow", bufs=4) as prow_p,
            tc.tile_pool(name="pt", bufs=1) as pt_p,
            tc.tile_pool(name="attnT", bufs=2) as attnT_p,
            tc.tile_pool(name="small", bufs=4) as small,
            tc.tile_pool(name="ost", bufs=2) as ost_p,
            # PSUM budget (8 banks): qko 2 + ss 4 + attnT 2
            tc.tile_pool(name="ps1", bufs=2, space="PSUM") as ps1,
            tc.tile_pool(name="psS", bufs=5, space="PSUM") as psS,
            tc.tile_pool(name="psa", bufs=1, space="PSUM") as psa_p,
        ):
            # ---- constants / weights ----
            qw_w = cpool.tile([P, NDT, E2], dt.float32, tag="qw")
            nc.sync.dma_start(qw_w[:], qw_d.rearrange("(t p) e -> p t e", p=P))
            kw_w = cpool.tile([P, NDT, E2], dt.float32, tag="kw")
            nc.sync.dma_start(kw_w[:], kw_d.rearrange("(t p) e -> p t e", p=P))
            vw_w = cpool.tile([P, NDT, E2], dt.bfloat16, tag="vw")
            nc.sync.dma_start(vw_w[:], vw_d.rearrange("(t p) e -> p t e", p=P))
            ow_w = cpool.tile([P, D], dt.bfloat16, tag="ow")
            nc.sync.dma_start(ow_w[:], ow_d)
            cm_sb = cpool.tile([P, 4, KB], dt.bfloat16, tag="cm")
            nc.sync.dma_start(cm_sb[:], me_d)

            # ---- per-batch: projections, then attention, then o_proj ----
            for _rep, b in __import__('itertools').product(range(reps), range(B)):
                QT = big.tile([P, S], dt.float32, tag="QT")
                KT = big.tile([P, S], dt.float32, tag="KT")
                Vs = big.tile([P, NST, E2], dt.bfloat16, tag="Vs")
                for sb in range(S // SBLK):
                    ssl = slice(sb * SBLK, (sb + 1) * SBLK)
                    x_t = xs.tile([P, NDT, SBLK], dt.float32, tag="x")
                    xh_t = xsh.tile([P, NDT, SBLK], dt.bfloat16, tag="xhb")
                    # halved loads so probs transposes can interleave sooner in
                    # the DMA stream (big copies block the xbar-transpose path)
                    xsrc = x_d[b, :, ssl].rearrange("(t p) s -> p t s", p=P)
                    hsrc = xh_d[b, :, ssl].rearrange("(t p) s -> p t s", p=P)
                    for t in range(0, NDT, 4):
                        nc.gpsimd.dma_start(x_t[:, t:t + 4], xsrc[:, t:t + 4])
                    nc.gpsimd.dma_start(xh_t[:], hsrc)
                    for dst, w_w in ((QT, qw_w), (KT, kw_w)):
                        ps = ps1.tile([P, SBLK], dt.float32, tag="qko", name="qk")
                        for t in range(NDT):
                            nc.tensor.matmul(
                                ps[:], w_w[:, t], x_t[:, t],
                                start=(t == 0), stop=(t == NDT - 1),
                            )
                        nc.vector.tensor_copy(
                            out=dst[:, sb * SBLK:(sb + 1) * SBLK], in_=ps[:],
                        )
                    # V^T chunk (bf16), then DMA-transpose into Vs
                    psv = ps1.tile([P, SBLK], dt.float32, tag="qko", name="v")
                    for t in range(NDT):
                        nc.tensor.matmul(
                            psv[:], vw_w[:, t], xh_t[:, t],
                            start=(t == 0), stop=(t == NDT - 1),
                        )
                    vt_t = vt_p.tile([P, SBLK], dt.bfloat16, tag="vt")
                    nc.vector.tensor_copy(out=vt_t[:], in_=psv[:])
                    nc.sync.dma_start_transpose(
                        out=Vs[:, sb * (SBLK // P):(sb + 1) * (SBLK // P), :],
                        in_=vt_t[:],
                    )

                # ---- attention for this batch ----
                attnT_sb = attnT_p.tile([P, NST, E2], dt.bfloat16, tag="attnT")

                def emit_pv(g, pt):
                    # PV for q-group g, heads col-packed; probs are normalized
                    # so this directly yields attnT.
                    psa = psa_p.tile([P, 4 * P], dt.float32, tag="a", name="a")
                    nkt = (g + 1) * 4
                    for kt in range(nkt):
                        for h in range(HPC):
                            nc.tensor.matmul(
                                psa[h * E:(h + 1) * E, :],
                                Vs[:, kt, h * E:(h + 1) * E],
                                pt[h][:, kt, :],
                                start=(kt == 0), stop=(kt == nkt - 1),
                            )
                    nc.vector.tensor_copy(
                        out=attnT_sb[:, 4 * g:4 * (g + 1), :]
                            .rearrange("p a b -> p (a b)"),
                        in_=psa[:],
                    )

                pending = None
                for g in range(NQT // 4):
                    kext = g + 1  # causal extent of the whole group, in KB blocks
                    pt = [pt_p.tile([P, NST, 4 * P], dt.bfloat16, tag=f"pt{h}",
                                    name=f"pt{h}") for h in range(HPC)]
                    for h in range(HPC):
                        # zero the future-k region the transposes no longer
                        # cover: kt in [qt+1, 4g+4) for q-column j
                        for j in range(3):
                            qt = 4 * g + j
                            nc.vector.memset(
                                pt[h][:, qt + 1:(g + 1) * 4,
                                      j * P:(j + 1) * P], 0.0)
                    for j in range(4):
                        qt = 4 * g + j
                        nmb = [small.tile([P, 4], dt.float32, tag=f"nmb{h}",
                                          name=f"nmb{h}") for h in range(HPC)]
                        lbuf = [small.tile([P, 4], dt.float32, tag=f"lb{h}",
                                           name=f"lb{h}") for h in range(HPC)]
                        prow = [prow_p.tile([P, S], dt.bfloat16, tag=f"prow{h}",
                                            name=f"prow{h}") for h in range(HPC)]
                        # valid columns in the last (diagonal) block
                        vext = (j + 1) * P
                        for kb in range(kext):
                            diag = kb == kext - 1
                            nv = vext if diag else KB
                            for h in range(HPC):
                                pss = psS.tile([P, KB], dt.float32, tag="ss")
                                nc.tensor.matmul(
                                    pss[:, :nv],
                                    QT[h * E:(h + 1) * E,
                                       qt * P:(qt + 1) * P],
                                    KT[h * E:(h + 1) * E,
                                       kb * KB:kb * KB + nv],
                                    start=True, stop=True,
                                )
                                if diag:
                                    nc.vector.tensor_tensor(
                                        pss[:, :nv], pss[:, :nv],
                                        cm_sb[:, j, :nv],
                                        mybir.AluOpType.add,
                                    )
                                nc.vector.reduce_max(
                                    out=nmb[h][:, kb:kb + 1], in_=pss[:, :nv],
                                    axis=mybir.AxisListType.X, negate=True,
                                )
                                nc.scalar.activation(
                                    out=prow[h][:, kb * KB:kb * KB + nv],
                                    in_=pss[:, :nv], func=Exp,
                                    bias=nmb[h][:, kb:kb + 1], scale=1.0,
                                    accum_out=lbuf[h][:, kb:kb + 1],
                                )
                        for h in range(HPC):
                            lr_h = small.tile([P, 1], dt.float32, tag=f"lr{h}",
                                              name=f"lr{h}")
                            if kext == 1:
                                nc.vector.reciprocal(lr_h[:], lbuf[h][:, 0:1])
                                nc.vector.tensor_scalar_mul(
                                    prow[h][:, :vext], prow[h][:, :vext], lr_h[:])
                            else:
                                negm = small.tile([P, 1], dt.float32,
                                                  tag=f"negm{h}", name=f"negm{h}")
                                cbuf = small.tile([P, 4], dt.float32,
                                                  tag=f"cb{h}", name=f"cb{h}")
                                l_h = small.tile([P, 1], dt.float32,
                                                 tag=f"l{h}", name=f"l{h}")
                                nc.vector.tensor_reduce(
                                    out=negm[:], in_=nmb[h][:, :kext],
                                    op=mybir.AluOpType.min,
                                    axis=mybir.AxisListType.X,
                                )
                                # c_kb = exp(m_kb - m) = exp(-nmb_kb + negm)
                                nc.scalar.activation(
                                    out=cbuf[:, :kext], in_=nmb[h][:, :kext],
                                    func=Exp, bias=negm[:], scale=-1.0,
                                )
                                nc.vector.tensor_tensor(
                                    lbuf[h][:, :kext], lbuf[h][:, :kext],
                                    cbuf[:, :kext], mybir.AluOpType.mult,
                                )
                                nc.vector.reduce_sum(
                                    out=l_h[:], in_=lbuf[h][:, :kext],
                                    axis=mybir.AxisListType.X,
                                )
                                nc.vector.reciprocal(lr_h[:], l_h[:])
                                # normalized correction: prow *= c_kb / l
                                nc.vector.tensor_scalar_mul(
                                    cbuf[:, :kext], cbuf[:, :kext], lr_h[:])
                                for kb in range(kext):
                                    nv = vext if kb == kext - 1 else KB
                                    nc.vector.tensor_scalar_mul(
                                        prow[h][:, kb * KB:kb * KB + nv],
                                        prow[h][:, kb * KB:kb * KB + nv],
                                        cbuf[:, kb:kb + 1],
                                    )
                            nc.sync.dma_start_transpose(
                                out=pt[h][:, :qt + 1, j * P:(j + 1) * P],
                                in_=prow[h][:, :(qt + 1) * P],
                            )
                        if j == 0 and pending is not None:
                            emit_pv(*pending)
                            pending = None
                    pending = (g, pt)
                if pending is not None:
                    emit_pv(*pending)
                    pending = None
                # ---- phase C: partial output projection for this batch ----
                for st in range(NST):
                    for dhalf in range(2):
                        pso = ps1.tile([P, 512], dt.float32, tag="qko", name="o")
                        nc.tensor.matmul(
                            pso[:], attnT_sb[:, st, :],
                            ow_w[:, dhalf * 512:(dhalf + 1) * 512],
                            start=True, stop=True,
                        )
                        osb = ost_p.tile([P, 512], dt.float32, tag="ost")
                        nc.scalar.copy(out=osb[:], in_=pso[:])
                        nc.sync.dma_start(
                            out_d[b, st * P:(st + 1) * P,
                                  dhalf * 512:(dhalf + 1) * 512],
                            osb[:],
                        )
    _split_multiwaits(nc)
    return nc


def make_in_maps(in_feature, q_proj, k_proj, v_proj, o_proj):
    import ml_dtypes

    bf16 = ml_dtypes.bfloat16
    x = np.asarray(in_feature, np.float32)
    xT = np.ascontiguousarray(x.transpose(0, 2, 1))          # [B, D, S]
    xh = xT.astype(bf16)

    scale = np.float32(1.0 / np.sqrt(E))
    qw = np.asarray(q_proj, np.float32).reshape(H, E, D) * scale
    kw = np.asarray(k_proj, np.float32).reshape(H, E, D)
    vw = np.asarray(v_proj, np.float32).reshape(H, E, D)
    ow = np.asarray(o_proj, np.float32).reshape(D, H, E)

    # additive causal masks for the diagonal 512-col block: pattern j is for
    # q-tiles at offset j*128 within the block (valid: k_local <= j*128 + p)
    idx = np.arange(KB)[None, :]
    me = np.stack(
        [np.where(idx <= (j * P + np.arange(P))[:, None], 0.0, FMIN)
         for j in range(4)], axis=1,
    ).astype(bf16)

    in_maps = []
    for c in range(NCORES):
        sl = slice(HPC * c, HPC * (c + 1))
        qT = np.ascontiguousarray(qw[sl].reshape(E2, D).T)   # [D, E2]
        kT = np.ascontiguousarray(kw[sl].reshape(E2, D).T)
        vT = np.ascontiguousarray(vw[sl].reshape(E2, D).T)
        oT = np.ascontiguousarray(ow[:, sl, :].reshape(D, E2).T)  # [E2, D]
        in_maps.append({
            "x": xT, "xh": xh,
            "qw": qT, "kw": kT,
            "vw": vT.astype(bf16), "ow": oT.astype(bf16), "me": me,
        })
    return in_maps


def kernel(in_feature, q_proj, k_proj, v_proj, o_proj, _results_hook=None):
    from concourse.bass_utils import run_bass_kernel_spmd

    global _BUILT
    if _BUILT is None:
        _BUILT = build_nc()
    in_maps = make_in_maps(in_feature, q_proj, k_proj, v_proj, o_proj)
    res = run_bass_kernel_spmd(_BUILT, in_maps, core_ids=list(range(NCORES)))
    if _results_hook is not None:
        _results_hook(res)
    out = np.zeros((B, S, D), np.float32)
    for r in res.results:
        out += r["out"]
    return out



# revision 11
# speedup vs baseline: 1.1360x; 1.1360x over previous
"""Causal MHA (B=4, S=2048, D=1024, H=16) on 8 NeuronCores.

Sharding: tensor-parallel over heads — 2 heads per core. Each core computes
Q/K/V projections for its 2 heads over the whole batch, causal flash-style
attention, and its partial output projection; the host sums the 8 partials.

Precision: the softmax here is near-argmax (scores have std ~1e3), so the
score path (Q/K/V projections and Q.K^T) runs in fp16 on the PE array
(10-bit mantissa at full 1-cycle/row rate; fp32 is 4x slower and fp32r
only has an 8-bit mantissa); probs/V/attn run fp16 as well, and the
per-core output partial is stored bf16 (summed in fp32 on the host).
Softmax is per-block online
(block max + per-block exp from PSUM + fused correction*1/l multiply), so the
PV matmul directly produces normalized attn^T with both heads column-packed.
"""

import numpy as np

B, S, D = 4, 2048, 1024
H, E = 16, 64
NCORES = 8
HPC = H // NCORES       # heads per core = 2
E2 = HPC * E            # 128
P = 128
KB = 512                # score block columns (k per psum tile)
SBLK = 512              # s-block for Q/K projection rhs tiles
NDT = D // P            # 8 d-tiles
NQT = S // P            # 16 q-tiles per batch
NST = S // P            # 16 s-tiles per batch
FMIN = -3.0e38

_BUILT = None


def _apply_drain_patch():
    """This walrus build rejects >1 sync-wait on a CTRL (Drain) instruction.
    Split the TileContext exit-drain waits across several drain instructions."""
    import concourse.tile as tile
    import concourse.mybir as mybir
    from concourse.vector_clock import ScopedClock

    if getattr(tile.TileContext, "_drain_patch_applied", False):
        return

    def _drain_and_barrier(self, tick_clock, wait_clock):
        nc = self.nc
        drain_inst = nc.sync.drain()
        wait_clock.add_sem_waits(
            drain_inst.ins, ScopedClock({None: tick_clock.global_clock})
        )
        si = drain_inst.ins.sync_info
        if si is not None and len(si.on_wait) > 1:
            waits = list(si.on_wait)
            del si.on_wait[1:]
            for w in waits[1:]:
                d2 = nc.sync.drain()
                d2.ins.sync_info = mybir.SyncInfo(on_wait=[w], on_update=[])
        nc.all_engine_barrier()
        popped = nc._tile_sem_poison_stack.pop()
        assert popped is self._sem_poison
        nc.clear_and_free_semaphores(list(self.sems.allocated().values()))
        nc.all_engine_barrier()

    tile.TileContext._drain_and_barrier = _drain_and_barrier
    tile.TileContext._drain_patch_applied = True


def _split_multiwaits(nc, max_waits=1):
    """This walrus build rejects instructions carrying more than ~1 sync-wait.
    Hoist extra waits onto single-wait NoOps on the same engine, placed just
    before the instruction (sequencers execute in order, so semantics hold)."""
    import concourse.mybir as mybir

    n_new = 0
    for f in nc.m.functions:
        for blk in f.blocks:
            insts = list(blk.instructions)
            if not any(
                getattr(i, "sync_info", None) is not None
                and len(i.sync_info.on_wait) > max_waits
                for i in insts
            ):
                continue
            out = []
            for inst in insts:
                si = getattr(inst, "sync_info", None)
                if si is not None and len(si.on_wait) > max_waits:
                    extra = list(si.on_wait[max_waits:])
                    del si.on_wait[max_waits:]
                    for w in extra:
                        n_new += 1
                        out.append(mybir.InstNoOp(
                            name=f"splitw-{n_new}",
                            sync_info=mybir.SyncInfo(on_wait=[w], on_update=[]),
                            engine=inst.engine,
                            bass_nofuse=True,
                        ))
                out.append(inst)
            blk.instructions[:] = out
    return n_new


def build_nc(reps=1):
    import concourse.bass as bass
    import concourse.mybir as mybir
    import concourse.tile as tile

    _apply_drain_patch()
    dt = mybir.dt
    Exp = mybir.ActivationFunctionType.Exp

    nc = bass.Bass("TRN2", target_bir_lowering=False, debug=False)

    x_d = nc.dram_tensor("x", [B, D, S], dt.float16, kind="ExternalInput").ap()
    qw_d = nc.dram_tensor("qw", [D, E2], dt.float16, kind="ExternalInput").ap()
    kw_d = nc.dram_tensor("kw", [D, E2], dt.float16, kind="ExternalInput").ap()
    vw_d = nc.dram_tensor("vw", [D, E2], dt.float16, kind="ExternalInput").ap()
    ow_d = nc.dram_tensor("ow", [E2, D], dt.float16, kind="ExternalInput").ap()
    me_d = nc.dram_tensor("me", [P, 4, KB], dt.bfloat16, kind="ExternalInput").ap()
    out_d = nc.dram_tensor("out", [B, S, D], dt.bfloat16, kind="ExternalOutput").ap()

    with tile.TileContext(nc) as tc:
        with (
            tc.tile_pool(name="const", bufs=1) as cpool,
            tc.tile_pool(name="big", bufs=2) as big,
            tc.tile_pool(name="xs", bufs=2) as xs,
            tc.tile_pool(name="vt", bufs=1) as vt_p,
            tc.tile_pool(name="prow", bufs=4) as prow_p,
            tc.tile_pool(name="pt", bufs=1) as pt_p,
            tc.tile_pool(name="attnT", bufs=2) as attnT_p,
            tc.tile_pool(name="small", bufs=4) as small,
            tc.tile_pool(name="ost", bufs=2) as ost_p,
            # PSUM budget (8 banks): qko 2 + ss 4 + attnT 2
            tc.tile_pool(name="ps1", bufs=2, space="PSUM") as ps1,
            tc.tile_pool(name="psS", bufs=5, space="PSUM") as psS,
            tc.tile_pool(name="psa", bufs=1, space="PSUM") as psa_p,
        ):
            # ---- constants / weights ----
            qw_w = cpool.tile([P, NDT, E2], dt.float16, tag="qw")
            nc.sync.dma_start(qw_w[:], qw_d.rearrange("(t p) e -> p t e", p=P))
            kw_w = cpool.tile([P, NDT, E2], dt.float16, tag="kw")
            nc.sync.dma_start(kw_w[:], kw_d.rearrange("(t p) e -> p t e", p=P))
            vw_w = cpool.tile([P, NDT, E2], dt.float16, tag="vw")
            nc.sync.dma_start(vw_w[:], vw_d.rearrange("(t p) e -> p t e", p=P))
            ow_w = cpool.tile([P, D], dt.float16, tag="ow")
            nc.sync.dma_start(ow_w[:], ow_d)
            cm_sb = cpool.tile([P, 4, KB], dt.bfloat16, tag="cm")
            nc.sync.dma_start(cm_sb[:], me_d)

            # ---- software-pipelined driver over (reps*B) logical batches:
            # super-iteration bb emits, per 512-col group sb:
            #   flush pv of the previous attn group (PE drains old work),
            #   attention group sb of batch bb-1 (DVE/Act-heavy),
            #   projection chunk sb of batch bb (PE/DMA-heavy).
            # This overlaps batch bb's projections with bb-1's softmax.
            NB = reps * B
            state = {}
            pending = [None]

            def emit_proj(bb, sb):
                b = bb % B
                if sb == 0:
                    state[bb] = {
                        "QT": big.tile([P, S], dt.float16, tag="QT"),
                        "KT": big.tile([P, S], dt.float16, tag="KT"),
                        "Vs": big.tile([P, NST, E2], dt.float16, tag="Vs"),
                        "attnT": attnT_p.tile([P, NST, E2], dt.float16,
                                              tag="attnT"),
                    }
                st = state[bb]
                QT, KT, Vs = st["QT"], st["KT"], st["Vs"]
                ssl = slice(sb * SBLK, (sb + 1) * SBLK)
                x_t = xs.tile([P, NDT, SBLK], dt.float16, tag="x")
                # halved loads so probs transposes can interleave sooner in
                # the DMA stream (big copies block the xbar-transpose path)
                xsrc = x_d[b, :, ssl].rearrange("(t p) s -> p t s", p=P)
                for t in range(0, NDT, 4):
                    nc.gpsimd.dma_start(x_t[:, t:t + 4], xsrc[:, t:t + 4])
                for dst, w_w in ((QT, qw_w), (KT, kw_w)):
                    ps = ps1.tile([P, SBLK], dt.float32, tag="qko", name="qk")
                    for t in range(NDT):
                        nc.tensor.matmul(
                            ps[:], w_w[:, t], x_t[:, t],
                            start=(t == 0), stop=(t == NDT - 1),
                        )
                    nc.vector.tensor_copy(
                        out=dst[:, sb * SBLK:(sb + 1) * SBLK], in_=ps[:],
                    )
                # V^T chunk, then DMA-transpose into Vs
                psv = ps1.tile([P, SBLK], dt.float32, tag="qko", name="v")
                for t in range(NDT):
                    nc.tensor.matmul(
                        psv[:], vw_w[:, t], x_t[:, t],
                        start=(t == 0), stop=(t == NDT - 1),
                    )
                vt_t = vt_p.tile([P, SBLK], dt.float16, tag="vt")
                nc.vector.tensor_copy(out=vt_t[:], in_=psv[:])
                nc.sync.dma_start_transpose(
                    out=Vs[:, sb * (SBLK // P):(sb + 1) * (SBLK // P), :],
                    in_=vt_t[:],
                )

            def emit_pv_flush():
                # PV for the pending q-group, heads col-packed; probs are
                # normalized so this directly yields attnT. Follows with the
                # o_proj chunks this group unblocks.
                if pending[0] is None:
                    return
                bb, g, pt = pending[0]
                pending[0] = None
                st = state[bb]
                Vs, attnT_sb = st["Vs"], st["attnT"]
                b = bb % B
                psa = psa_p.tile([P, 4 * P], dt.float32, tag="a", name="a")
                nkt = (g + 1) * 4
                for kt in range(nkt):
                    for h in range(HPC):
                        nc.tensor.matmul(
                            psa[h * E:(h + 1) * E, :],
                            Vs[:, kt, h * E:(h + 1) * E],
                            pt[h][:, kt, :],
                            start=(kt == 0), stop=(kt == nkt - 1),
                        )
                nc.vector.tensor_copy(
                    out=attnT_sb[:, 4 * g:4 * (g + 1), :]
                        .rearrange("p a b -> p (a b)"),
                    in_=psa[:],
                )
                # partial output projection for the q-tiles of this group
                for qt in range(4 * g, 4 * g + 4):
                    for dhalf in range(2):
                        pso = ps1.tile([P, 512], dt.float32, tag="qko",
                                       name="o")
                        nc.tensor.matmul(
                            pso[:], attnT_sb[:, qt, :],
                            ow_w[:, dhalf * 512:(dhalf + 1) * 512],
                            start=True, stop=True,
                        )
                        osb = ost_p.tile([P, 512], dt.bfloat16, tag="ost")
                        nc.scalar.copy(out=osb[:], in_=pso[:])
                        nc.sync.dma_start(
                            out_d[b, qt * P:(qt + 1) * P,
                                  dhalf * 512:(dhalf + 1) * 512],
                            osb[:],
                        )
                if g == NQT // 4 - 1:
                    del state[bb]

            def emit_attn_group(bb, g):
                st = state[bb]
                QT, KT = st["QT"], st["KT"]
                kext = g + 1  # causal extent of the group, in KB blocks
                pt = [pt_p.tile([P, NST, 4 * P], dt.float16, tag=f"pt{h}",
                                name=f"pt{h}") for h in range(HPC)]
                for h in range(HPC):
                    # zero the future-k region the transposes no longer
                    # cover: kt in [qt+1, 4g+4) for q-column j
                    for j in range(3):
                        qt = 4 * g + j
                        nc.gpsimd.memset(
                            pt[h][:, qt + 1:(g + 1) * 4,
                                  j * P:(j + 1) * P], 0.0)
                for j in range(4):
                    qt = 4 * g + j
                    nmb = [small.tile([P, 4], dt.float32, tag=f"nmb{h}",
                                      name=f"nmb{h}") for h in range(HPC)]
                    lbuf = [small.tile([P, 4], dt.float32, tag=f"lb{h}",
                                       name=f"lb{h}") for h in range(HPC)]
                    prow = [prow_p.tile([P, S], dt.float16, tag=f"prow{h}",
                                        name=f"prow{h}") for h in range(HPC)]
                    # valid columns in the last (diagonal) block
                    vext = (j + 1) * P
                    for kb in range(kext):
                        diag = kb == kext - 1
                        nv = vext if diag else KB
                        for h in range(HPC):
                            pss = psS.tile([P, KB], dt.float32, tag="ss")
                            nc.tensor.matmul(
                                pss[:, :nv],
                                QT[h * E:(h + 1) * E,
                                   qt * P:(qt + 1) * P],
                                KT[h * E:(h + 1) * E,
                                   kb * KB:kb * KB + nv],
                                start=True, stop=True,
                            )
                            if diag:
                                nc.vector.tensor_tensor(
                                    pss[:, :nv], pss[:, :nv],
                                    cm_sb[:, j, :nv],
                                    mybir.AluOpType.add,
                                )
                            nc.vector.reduce_max(
                                out=nmb[h][:, kb:kb + 1], in_=pss[:, :nv],
                                axis=mybir.AxisListType.X, negate=True,
                            )
                            nc.scalar.activation(
                                out=prow[h][:, kb * KB:kb * KB + nv],
                                in_=pss[:, :nv], func=Exp,
                                bias=nmb[h][:, kb:kb + 1], scale=1.0,
                                accum_out=lbuf[h][:, kb:kb + 1],
                            )
                    for h in range(HPC):
                        lr_h = small.tile([P, 1], dt.float32, tag=f"lr{h}",
                                          name=f"lr{h}")
                        if kext == 1:
                            nc.vector.reciprocal(lr_h[:], lbuf[h][:, 0:1])
                            nc.vector.tensor_scalar_mul(
                                prow[h][:, :vext], prow[h][:, :vext], lr_h[:])
                        else:
                            negm = small.tile([P, 1], dt.float32,
                                              tag=f"negm{h}", name=f"negm{h}")
                            cbuf = small.tile([P, 4], dt.float32,
                                              tag=f"cb{h}", name=f"cb{h}")
                            l_h = small.tile([P, 1], dt.float32,
                                             tag=f"l{h}", name=f"l{h}")
                            nc.vector.tensor_reduce(
                                out=negm[:], in_=nmb[h][:, :kext],
                                op=mybir.AluOpType.min,
                                axis=mybir.AxisListType.X,
                            )
                            # c_kb = exp(m_kb - m) = exp(-nmb_kb + negm)
                            nc.scalar.activation(
                                out=cbuf[:, :kext], in_=nmb[h][:, :kext],
                                func=Exp, bias=negm[:], scale=-1.0,
                            )
                            nc.vector.tensor_tensor(
                                lbuf[h][:, :kext], lbuf[h][:, :kext],
                                cbuf[:, :kext], mybir.AluOpType.mult,
                            )
                            nc.vector.reduce_sum(
                                out=l_h[:], in_=lbuf[h][:, :kext],
                                axis=mybir.AxisListType.X,
                            )
                            nc.vector.reciprocal(lr_h[:], l_h[:])
                            # normalized correction: prow *= c_kb / l
                            nc.vector.tensor_scalar_mul(
                                cbuf[:, :kext], cbuf[:, :kext], lr_h[:])
                            for kb in range(kext):
                                nv = vext if kb == kext - 1 else KB
                                nc.vector.tensor_scalar_mul(
                                    prow[h][:, kb * KB:kb * KB + nv],
                                    prow[h][:, kb * KB:kb * KB + nv],
                                    cbuf[:, kb:kb + 1],
                                )
                        nc.sync.dma_start_transpose(
                            out=pt[h][:, :qt + 1, j * P:(j + 1) * P],
                            in_=prow[h][:, :(qt + 1) * P],
                        )
                pending[0] = (bb, g, pt)

            for bb in range(NB + 1):
                for sb in range(4):
                    emit_pv_flush()
                    if bb >= 1:
                        emit_attn_group(bb - 1, sb)
                    if bb < NB:
                        emit_proj(bb, sb)
            emit_pv_flush()
    _split_multiwaits(nc)
    return nc


def make_in_maps(in_feature, q_proj, k_proj, v_proj, o_proj):
    import ml_dtypes

    bf16 = ml_dtypes.bfloat16
    x = np.asarray(in_feature, np.float32)
    xh = np.ascontiguousarray(x.transpose(0, 2, 1)).astype(np.float16)

    scale = np.float32(1.0 / np.sqrt(E))
    qw = np.asarray(q_proj, np.float32).reshape(H, E, D) * scale
    kw = np.asarray(k_proj, np.float32).reshape(H, E, D)
    vw = np.asarray(v_proj, np.float32).reshape(H, E, D)
    ow = np.asarray(o_proj, np.float32).reshape(D, H, E)

    # additive causal masks for the diagonal 512-col block: pattern j is for
    # q-tiles at offset j*128 within the block (valid: k_local <= j*128 + p)
    idx = np.arange(KB)[None, :]
    me = np.stack(
        [np.where(idx <= (j * P + np.arange(P))[:, None], 0.0, FMIN)
         for j in range(4)], axis=1,
    ).astype(bf16)

    in_maps = []
    for c in range(NCORES):
        sl = slice(HPC * c, HPC * (c + 1))
        qT = np.ascontiguousarray(qw[sl].reshape(E2, D).T)   # [D, E2]
        kT = np.ascontiguousarray(kw[sl].reshape(E2, D).T)
        vT = np.ascontiguousarray(vw[sl].reshape(E2, D).T)
        oT = np.ascontiguousarray(ow[:, sl, :].reshape(D, E2).T)  # [E2, D]
        in_maps.append({
            "x": xh,
            "qw": qT.astype(np.float16), "kw": kT.astype(np.float16),
            "vw": vT.astype(np.float16), "ow": oT.astype(np.float16),
            "me": me,
        })
    return in_maps


def kernel(in_feature, q_proj, k_proj, v_proj, o_proj, _results_hook=None):
    from concourse.bass_utils import run_bass_kernel_spmd

    global _BUILT
    if _BUILT is None:
        _BUILT = build_nc()
    in_maps = make_in_maps(in_feature, q_proj, k_proj, v_proj, o_proj)
    res = run_bass_kernel_spmd(_BUILT, in_maps, core_ids=list(range(NCORES)))
    if _results_hook is not None:
        _results_hook(res)
    out = np.zeros((B, S, D), np.float32)
    for r in res.results:
        out += np.asarray(r["out"], np.float32)
    return out



# revision 34
# speedup vs baseline: 1.2399x; 1.0915x over previous
"""Causal MHA (B=4, S=2048, D=1024, H=16) on 8 NeuronCores.

Sharding: tensor-parallel over heads — 2 heads per core. Each core computes
Q/K/V projections for its 2 heads over the whole batch, causal flash-style
attention, and its partial output projection; the host sums the 8 partials.

Precision: the softmax here is near-argmax (scores have std ~1e3), so the
score path runs on fp16 hi+lo splits of x and the q/k weights
(3-term split matmuls at the full 1-cycle/row fp16 rate; fp32 is 4x
slower and fp32r only has an 8-bit mantissa), with Q/K themselves
stored single-fp16; probs/V/attn run fp16, and the per-core output
partial is stored fp16 (summed in fp32 on the host).
Softmax is per-block online
(block max + per-block exp from PSUM + fused correction*1/l multiply), so the
PV matmul directly produces normalized attn^T with both heads column-packed.
"""

import numpy as np

B, S, D = 4, 2048, 1024
H, E = 16, 64
NCORES = 8
HPC = H // NCORES       # heads per core = 2
E2 = HPC * E            # 128
P = 128
KB = 512                # score block columns (k per psum tile)
SBLK = 512              # s-block for Q/K projection rhs tiles
NDT = D // P            # 8 d-tiles
NQT = S // P            # 16 q-tiles per batch
NST = S // P            # 16 s-tiles per batch
FMIN = -3.0e38

_BUILT = None


def _apply_drain_patch():
    """This walrus build rejects >1 sync-wait on a CTRL (Drain) instruction.
    Split the TileContext exit-drain waits across several drain instructions."""
    import concourse.tile as tile
    import concourse.mybir as mybir
    from concourse.vector_clock import ScopedClock

    if getattr(tile.TileContext, "_drain_patch_applied", False):
        return

    def _drain_and_barrier(self, tick_clock, wait_clock):
        nc = self.nc
        drain_inst = nc.sync.drain()
        wait_clock.add_sem_waits(
            drain_inst.ins, ScopedClock({None: tick_clock.global_clock})
        )
        si = drain_inst.ins.sync_info
        if si is not None and len(si.on_wait) > 1:
            waits = list(si.on_wait)
            del si.on_wait[1:]
            for w in waits[1:]:
                d2 = nc.sync.drain()
                d2.ins.sync_info = mybir.SyncInfo(on_wait=[w], on_update=[])
        nc.all_engine_barrier()
        popped = nc._tile_sem_poison_stack.pop()
        assert popped is self._sem_poison
        nc.clear_and_free_semaphores(list(self.sems.allocated().values()))
        nc.all_engine_barrier()

    tile.TileContext._drain_and_barrier = _drain_and_barrier
    tile.TileContext._drain_patch_applied = True


def _split_multiwaits(nc, max_waits=1):
    """This walrus build rejects instructions carrying more than ~1 sync-wait.
    Hoist extra waits onto single-wait NoOps on the same engine, placed just
    before the instruction (sequencers execute in order, so semantics hold)."""
    import concourse.mybir as mybir

    n_new = 0
    for f in nc.m.functions:
        for blk in f.blocks:
            insts = list(blk.instructions)
            if not any(
                getattr(i, "sync_info", None) is not None
                and len(i.sync_info.on_wait) > max_waits
                for i in insts
            ):
                continue
            out = []
            for inst in insts:
                si = getattr(inst, "sync_info", None)
                if si is not None and len(si.on_wait) > max_waits:
                    extra = list(si.on_wait[max_waits:])
                    del si.on_wait[max_waits:]
                    for w in extra:
                        n_new += 1
                        out.append(mybir.InstNoOp(
                            name=f"splitw-{n_new}",
                            sync_info=mybir.SyncInfo(on_wait=[w], on_update=[]),
                            engine=inst.engine,
                            bass_nofuse=True,
                        ))
                out.append(inst)
            blk.instructions[:] = out
    return n_new


def build_nc(reps=1):
    import concourse.bass as bass
    import concourse.mybir as mybir
    import concourse.tile as tile

    _apply_drain_patch()
    dt = mybir.dt
    Exp = mybir.ActivationFunctionType.Exp

    nc = bass.Bass("TRN2", target_bir_lowering=False, debug=False)

    x_d = nc.dram_tensor("x", [B, D, S], dt.float16, kind="ExternalInput").ap()
    xl_d = nc.dram_tensor("xl", [B, D, S], dt.float16, kind="ExternalInput").ap()
    qw_d = nc.dram_tensor("qw", [D, E2], dt.float16, kind="ExternalInput").ap()
    qwl_d = nc.dram_tensor("qwl", [D, E2], dt.float16, kind="ExternalInput").ap()
    kw_d = nc.dram_tensor("kw", [D, E2], dt.float16, kind="ExternalInput").ap()
    kwl_d = nc.dram_tensor("kwl", [D, E2], dt.float16, kind="ExternalInput").ap()
    vw_d = nc.dram_tensor("vw", [D, E2], dt.float16, kind="ExternalInput").ap()
    ow_d = nc.dram_tensor("ow", [E2, D], dt.float16, kind="ExternalInput").ap()
    me_d = nc.dram_tensor("me", [P, 4, KB], dt.bfloat16, kind="ExternalInput").ap()
    out_d = nc.dram_tensor("out", [B, S, D], dt.float16, kind="ExternalOutput").ap()

    with tile.TileContext(nc) as tc:
        with (
            tc.tile_pool(name="const", bufs=1) as cpool,
            tc.tile_pool(name="big", bufs=2) as big,
            tc.tile_pool(name="xs", bufs=2) as xs,
            tc.tile_pool(name="vt", bufs=2) as vt_p,
            tc.tile_pool(name="prow", bufs=6) as prow_p,
            tc.tile_pool(name="pt", bufs=2) as pt_p,
            tc.tile_pool(name="attnT", bufs=2) as attnT_p,
            tc.tile_pool(name="small", bufs=8) as small,
            tc.tile_pool(name="ost", bufs=4) as ost_p,
            # PSUM budget (8 banks): qko 2 + ss 5 + attnT 1
            tc.tile_pool(name="ps1", bufs=2, space="PSUM") as ps1,
            tc.tile_pool(name="psS", bufs=5, space="PSUM") as psS,
            tc.tile_pool(name="psa", bufs=1, space="PSUM") as psa_p,
        ):
            # ---- constants / weights ----
            qw_w = cpool.tile([P, NDT, E2], dt.float16, tag="qw")
            nc.sync.dma_start(qw_w[:], qw_d.rearrange("(t p) e -> p t e", p=P))
            kw_w = cpool.tile([P, NDT, E2], dt.float16, tag="kw")
            nc.sync.dma_start(kw_w[:], kw_d.rearrange("(t p) e -> p t e", p=P))
            qwl_w = cpool.tile([P, NDT, E2], dt.float16, tag="qwl")
            nc.sync.dma_start(qwl_w[:], qwl_d.rearrange("(t p) e -> p t e", p=P))
            kwl_w = cpool.tile([P, NDT, E2], dt.float16, tag="kwl")
            nc.sync.dma_start(kwl_w[:], kwl_d.rearrange("(t p) e -> p t e", p=P))
            vw_w = cpool.tile([P, NDT, E2], dt.float16, tag="vw")
            nc.sync.dma_start(vw_w[:], vw_d.rearrange("(t p) e -> p t e", p=P))
            ow_w = cpool.tile([P, D], dt.float16, tag="ow")
            nc.sync.dma_start(ow_w[:], ow_d)
            # mask duplicated per head so one tensor_tensor covers the
            # head-paired score tile
            cm_sb = cpool.tile([P, 2, 4, KB], dt.bfloat16, tag="cm")
            nc.sync.dma_start(cm_sb[:, 0], me_d)
            nc.sync.dma_start(cm_sb[:, 1], me_d)

            # ---- software-pipelined driver over (reps*B) logical batches:
            # super-iteration bb emits, per 512-col group sb:
            #   flush pv of the previous attn group (PE drains old work),
            #   attention group sb of batch bb-1 (DVE/Act-heavy),
            #   projection chunk sb of batch bb (PE/DMA-heavy).
            # This overlaps batch bb's projections with bb-1's softmax.
            NB = reps * B
            state = {}
            pending = [None]

            def emit_xload(bb, sb):
                b = bb % B
                if sb == 0:
                    state[bb] = {
                        "QT": big.tile([P, S], dt.float16, tag="QT", name="QT"),
                        "KT": big.tile([P, S], dt.float16, tag="KT", name="KT"),
                        "Vs": big.tile([P, NST, E2], dt.float16, tag="Vs", name="Vs"),
                        "attnT": attnT_p.tile([P, NST, E2], dt.float16,
                                              tag="attnT", name="attnT"),
                    }
                st = state[bb]
                ssl = slice(sb * SBLK, (sb + 1) * SBLK)
                x_t = xs.tile([P, NDT, SBLK], dt.float16, tag="x")
                xl_t = xs.tile([P, NDT, SBLK], dt.float16, tag="xl")
                # halved loads so probs transposes can interleave sooner in
                # the DMA stream (big copies block the xbar-transpose path)
                xsrc = x_d[b, :, ssl].rearrange("(t p) s -> p t s", p=P)
                xlsrc = xl_d[b, :, ssl].rearrange("(t p) s -> p t s", p=P)
                for t in range(0, NDT, 4):
                    nc.gpsimd.dma_start(x_t[:, t:t + 4], xsrc[:, t:t + 4])
                    nc.gpsimd.dma_start(xl_t[:, t:t + 4], xlsrc[:, t:t + 4])
                st["x_t"], st["xl_t"] = x_t, xl_t

            def emit_proj(bb, sb):
                st = state[bb]
                QT, KT, Vs = st["QT"], st["KT"], st["Vs"]
                x_t, xl_t = st.pop("x_t"), st.pop("xl_t")
                # hi/lo split: Q = Wh xh + Wh xl + Wl xh (the dropped Wl xl
                # term is ~2^-22 relative); needed because the huge score
                # scale (std ~1e3) amplifies any single-fp16 rounding of the
                # inputs into O(1) score error.
                for dst, w_w, wl_w in ((QT, qw_w, qwl_w), (KT, kw_w, kwl_w)):
                    ps = ps1.tile([P, SBLK], dt.float32, tag="qko", name="qk")
                    for t in range(NDT):
                        nc.tensor.matmul(
                            ps[:], w_w[:, t], x_t[:, t],
                            start=(t == 0), stop=False,
                        )
                        nc.tensor.matmul(
                            ps[:], w_w[:, t], xl_t[:, t],
                            start=False, stop=False,
                        )
                        nc.tensor.matmul(
                            ps[:], wl_w[:, t], x_t[:, t],
                            start=False, stop=(t == NDT - 1),
                        )
                    nc.vector.tensor_copy(
                        out=dst[:, sb * SBLK:(sb + 1) * SBLK], in_=ps[:],
                    )
                # V^T chunk, then DMA-transpose into Vs
                psv = ps1.tile([P, SBLK], dt.float32, tag="qko", name="v")
                for t in range(NDT):
                    nc.tensor.matmul(
                        psv[:], vw_w[:, t], x_t[:, t],
                        start=(t == 0), stop=(t == NDT - 1),
                    )
                vt_t = vt_p.tile([P, SBLK], dt.float16, tag="vt")
                nc.vector.tensor_copy(out=vt_t[:], in_=psv[:])
                nc.sync.dma_start_transpose(
                    out=Vs[:, sb * (SBLK // P):(sb + 1) * (SBLK // P), :],
                    in_=vt_t[:],
                )

            def emit_pv_flush():
                # PV for the pending q-group, heads col-packed; probs are
                # normalized so this directly yields attnT. Follows with the
                # o_proj chunks this group unblocks.
                if pending[0] is None:
                    return
                bb, g, pt = pending[0]
                pending[0] = None
                st = state[bb]
                Vs, attnT_sb = st["Vs"], st["attnT"]
                b = bb % B
                psa = psa_p.tile([P, 4 * P], dt.float32, tag="a", name="a")
                nkt = (g + 1) * 4
                for kt in range(nkt):
                    for h in range(HPC):
                        nc.tensor.matmul(
                            psa[h * E:(h + 1) * E, :],
                            Vs[:, kt, h * E:(h + 1) * E],
                            pt[h][:, kt, :],
                            start=(kt == 0), stop=(kt == nkt - 1),
                        )
                nc.vector.tensor_copy(
                    out=attnT_sb[:, 4 * g:4 * (g + 1), :]
                        .rearrange("p a b -> p (a b)"),
                    in_=psa[:],
                )
                # partial output projection for the q-tiles of this group;
                # the PSUM->SBUF fp16 bounce runs on the (mostly idle) Pool
                # engine to keep Act free for the exps
                for qt in range(4 * g, 4 * g + 4):
                    for dhalf in range(2):
                        pso = ps1.tile([P, 512], dt.float32, tag="qko",
                                       name="o")
                        nc.tensor.matmul(
                            pso[:], attnT_sb[:, qt, :],
                            ow_w[:, dhalf * 512:(dhalf + 1) * 512],
                            start=True, stop=True,
                        )
                        osb = ost_p.tile([P, 512], dt.float16, tag="ost")
                        nc.scalar.copy(out=osb[:], in_=pso[:])
                        nc.sync.dma_start(
                            out_d[b, qt * P:(qt + 1) * P,
                                  dhalf * 512:(dhalf + 1) * 512],
                            osb[:],
                        )
                if g == NQT // 4 - 1:
                    del state[bb]

            def emit_attn_group(bb, g):
                st = state[bb]
                QT, KT = st["QT"], st["KT"]
                kext = g + 1  # causal extent of the group, in KB blocks
                pt = [pt_p.tile([P, NST, 4 * P], dt.float16, tag=f"pt{h}",
                                name=f"pt{h}") for h in range(HPC)]
                for h in range(HPC):
                    # zero the future-k region the transposes no longer
                    # cover: kt in [qt+1, 4g+4) for q-column j
                    for j in range(3):
                        qt = 4 * g + j
                        nc.gpsimd.memset(
                            pt[h][:, qt + 1:(g + 1) * 4,
                                  j * P:(j + 1) * P], 0.0)
                for j in range(4):
                    qt = 4 * g + j
                    nmb2 = small.tile([P, 2, 4], dt.float32, tag="nmb2",
                                      name="nmb2")
                    lbuf = [small.tile([P, 4], dt.float32, tag=f"lb{h}",
                                       name=f"lb{h}") for h in range(HPC)]
                    prow = [prow_p.tile([P, S], dt.float16, tag=f"prow{h}",
                                        name=f"prow{h}") for h in range(HPC)]
                    # valid columns in the last (diagonal) block
                    vext = (j + 1) * P
                    for kb in range(kext):
                        diag = kb == kext - 1
                        nv = vext if diag else KB
                        for h in range(HPC):
                            pss = psS.tile([P, KB], dt.float32, tag="ss")
                            nc.tensor.matmul(
                                pss[:, :nv],
                                QT[h * E:(h + 1) * E,
                                   qt * P:(qt + 1) * P],
                                KT[h * E:(h + 1) * E,
                                   kb * KB:kb * KB + nv],
                                start=True, stop=True,
                            )
                            if diag:
                                nc.vector.tensor_tensor(
                                    pss[:, :nv], pss[:, :nv],
                                    cm_sb[:, 0, j, :nv],
                                    mybir.AluOpType.add,
                                )
                            nc.vector.reduce_max(
                                out=nmb2[:, h, kb:kb + 1], in_=pss[:, :nv],
                                axis=mybir.AxisListType.X, negate=True,
                            )
                            nc.scalar.activation(
                                out=prow[h][:, kb * KB:kb * KB + nv],
                                in_=pss[:, :nv], func=Exp,
                                bias=nmb2[:, h, kb:kb + 1], scale=1.0,
                                accum_out=lbuf[h][:, kb:kb + 1],
                            )
                    for h in range(HPC):
                        lr_h = small.tile([P, 1], dt.float32, tag=f"lr{h}",
                                          name=f"lr{h}")
                        if kext == 1:
                            nc.vector.reciprocal(lr_h[:], lbuf[h][:, 0:1])
                            nc.vector.tensor_scalar_mul(
                                prow[h][:, :vext], prow[h][:, :vext], lr_h[:])
                        else:
                            negm = small.tile([P, 1], dt.float32,
                                              tag=f"negm{h}", name=f"negm{h}")
                            cbuf = small.tile([P, 4], dt.float32,
                                              tag=f"cb{h}", name=f"cb{h}")
                            l_h = small.tile([P, 1], dt.float32,
                                             tag=f"l{h}", name=f"l{h}")
                            nc.vector.tensor_reduce(
                                out=negm[:], in_=nmb2[:, h, :kext],
                                op=mybir.AluOpType.min,
                                axis=mybir.AxisListType.X,
                            )
                            # c_kb = exp(m_kb - m) = exp(-nmb_kb + negm)
                            nc.scalar.activation(
                                out=cbuf[:, :kext], in_=nmb2[:, h, :kext],
                                func=Exp, bias=negm[:], scale=-1.0,
                            )
                            nc.vector.tensor_tensor(
                                lbuf[h][:, :kext], lbuf[h][:, :kext],
                                cbuf[:, :kext], mybir.AluOpType.mult,
                            )
                            nc.vector.reduce_sum(
                                out=l_h[:], in_=lbuf[h][:, :kext],
                                axis=mybir.AxisListType.X,
                            )
                            nc.vector.reciprocal(lr_h[:], l_h[:])
                            # normalized correction: prow *= c_kb / l
                            nc.vector.tensor_scalar_mul(
                                cbuf[:, :kext], cbuf[:, :kext], lr_h[:])
                            for kb in range(kext):
                                nv = vext if kb == kext - 1 else KB
                                nc.vector.tensor_scalar_mul(
                                    prow[h][:, kb * KB:kb * KB + nv],
                                    prow[h][:, kb * KB:kb * KB + nv],
                                    cbuf[:, kb:kb + 1],
                                )
                        nc.sync.dma_start_transpose(
                            out=pt[h][:, :qt + 1, j * P:(j + 1) * P],
                            in_=prow[h][:, :(qt + 1) * P],
                        )
                pending[0] = (bb, g, pt)

            for bb in range(NB + 1):
                for sb in range(4):
                    emit_pv_flush()
                    if bb >= 1:
                        emit_attn_group(bb - 1, sb)
                    if bb < NB:
                        emit_xload(bb, sb)
                        emit_proj(bb, sb)
            emit_pv_flush()
    _split_multiwaits(nc)
    return nc


def make_in_maps(in_feature, q_proj, k_proj, v_proj, o_proj):
    import ml_dtypes

    bf16 = ml_dtypes.bfloat16
    f16 = np.float16
    x = np.asarray(in_feature, np.float32)
    xT = np.ascontiguousarray(x.transpose(0, 2, 1))
    xh = xT.astype(f16)
    xl = (xT - xh.astype(np.float32)).astype(f16)

    scale = np.float32(1.0 / np.sqrt(E))
    qw = np.asarray(q_proj, np.float32).reshape(H, E, D) * scale
    kw = np.asarray(k_proj, np.float32).reshape(H, E, D)
    vw = np.asarray(v_proj, np.float32).reshape(H, E, D)
    ow = np.asarray(o_proj, np.float32).reshape(D, H, E)

    # additive causal masks for the diagonal 512-col block: pattern j is for
    # q-tiles at offset j*128 within the block (valid: k_local <= j*128 + p)
    idx = np.arange(KB)[None, :]
    me = np.stack(
        [np.where(idx <= (j * P + np.arange(P))[:, None], 0.0, FMIN)
         for j in range(4)], axis=1,
    ).astype(bf16)

    in_maps = []
    for c in range(NCORES):
        sl = slice(HPC * c, HPC * (c + 1))
        qT = np.ascontiguousarray(qw[sl].reshape(E2, D).T)   # [D, E2]
        kT = np.ascontiguousarray(kw[sl].reshape(E2, D).T)
        vT = np.ascontiguousarray(vw[sl].reshape(E2, D).T)
        oT = np.ascontiguousarray(ow[:, sl, :].reshape(D, E2).T)  # [E2, D]
        qh = qT.astype(f16)
        kh = kT.astype(f16)
        in_maps.append({
            "x": xh, "xl": xl,
            "qw": qh, "qwl": (qT - qh.astype(np.float32)).astype(f16),
            "kw": kh, "kwl": (kT - kh.astype(np.float32)).astype(f16),
            "vw": vT.astype(f16), "ow": oT.astype(f16),
            "me": me,
        })
    return in_maps


def kernel(in_feature, q_proj, k_proj, v_proj, o_proj, _results_hook=None):
    from concourse.bass_utils import run_bass_kernel_spmd

    global _BUILT
    if _BUILT is None:
        _BUILT = build_nc()
    in_maps = make_in_maps(in_feature, q_proj, k_proj, v_proj, o_proj)
    res = run_bass_kernel_spmd(_BUILT, in_maps, core_ids=list(range(NCORES)))
    if _results_hook is not None:
        _results_hook(res)
    out = np.zeros((B, S, D), np.float32)
    for r in res.results:
        out += np.asarray(r["out"], np.float32)
    return out

